# revision 1
# baseline (speedup 1.0000x reference)
"""Trainium2 Bass kernel: frame-block-causal multi-head attention with LayerNorm.

Full module: LayerNorm(x) -> QKV proj -> 16-head block-causal attention
(8 frames x 256 patches) -> output projection.

Sharding: 8 cores = batch(2) x head-groups(4 heads each).  Each core gets its
batch's x and the weight column/row slices for its 4 heads, computes a partial
output [2048, 1024]; host sums the 4 partials per batch.  No collectives.

All matmuls run in bf16 with fp32 PSUM accumulation.  LayerNorm statistics are
computed in fp32.  g (LN gain) is folded into w_qkv on the host; LN beta's
qkv-bias is applied on-device for q/k and folded into a constant output bias
for v (softmax rows sum to 1).
"""

import numpy as np
import ml_dtypes

import concourse.bass as bass
import concourse.mybir as mybir
import concourse.tile as tile
from concourse import bacc, bass_utils

# ---- problem constants (hardcoded; kernel.py must be self-contained) ----
DIM = 1024
HEADS = 16
DIM_HEAD = 64
NUM_FRAMES = 8
PATCHES_PER_FRAME = 256
BATCH = 2
SEQ = NUM_FRAMES * PATCHES_PER_FRAME  # 2048
EPS = 1e-5
SCALE = DIM_HEAD ** -0.5  # 0.125

N_CORES = 8
HG = 4          # heads per core
NT = SEQ // 128   # 16 seq tiles of 128
NCK = DIM // 128  # 8 contraction chunks

dt = mybir.dt
AF = mybir.ActivationFunctionType
ALU = mybir.AluOpType


def _pieces(qlo):
    """512-aligned chunks covering [qlo, SEQ). qlo is a multiple of 256."""
    out = []
    p = qlo
    if p % 512:
        out.append((p, 256))
        p += 256
    while p < SEQ:
        out.append((p, 512))
        p += 512
    return out


def build_program():
    nc = bacc.Bacc("TRN2", target_bir_lowering=False, debug=False)
    f32, bf16 = dt.float32, dt.bfloat16

    x_d = nc.dram_tensor("x", [SEQ, DIM], f32, kind="ExternalInput")
    wq_d = nc.dram_tensor("wq", [DIM, HG * DIM_HEAD], bf16, kind="ExternalInput")
    wk_d = nc.dram_tensor("wk", [DIM, HG * DIM_HEAD], bf16, kind="ExternalInput")
    wv_d = nc.dram_tensor("wv", [DIM, HG * DIM_HEAD], bf16, kind="ExternalInput")
    wo_d = nc.dram_tensor("wo", [HG * DIM_HEAD, DIM], bf16, kind="ExternalInput")
    bqk_d = nc.dram_tensor("bqk", [128, 4], f32, kind="ExternalInput")
    out_d = nc.dram_tensor("out", [SEQ, DIM], f32, kind="ExternalOutput")

    with tile.TileContext(nc) as tc:
        _build(tc, nc, x_d, wq_d, wk_d, wv_d, wo_d, bqk_d, out_d)
    nc.compile()
    return nc


def _build(tc, nc, x_d, wq_d, wk_d, wv_d, wo_d, bqk_d, out_d):
    f32, bf16 = dt.float32, dt.bfloat16
    from concourse.masks import make_identity

    with tc.tile_pool(name="persist", bufs=1) as persist:
        _build_body(tc, nc, persist, x_d, wq_d, wk_d, wv_d, wo_d, bqk_d, out_d)


def _build_body(tc, nc, persist, x_d, wq_d, wk_d, wv_d, wo_d, bqk_d, out_d):
    f32, bf16 = dt.float32, dt.bfloat16
    from concourse.masks import make_identity

    # persistent SBUF tensors
    xhatT = persist.tile([128, NCK, SEQ], bf16)          # x-hat transposed, chunked
    wq_sb = persist.tile([128, NCK, HG * DIM_HEAD], bf16)
    wk_sb = persist.tile([128, NCK, HG * DIM_HEAD], bf16)
    wv_sb = persist.tile([128, NCK, HG * DIM_HEAD], bf16)
    wo_sb = persist.tile([128, 2, 2, 512], bf16)         # [pair-row, jc, half, n]
    bqk_sb = persist.tile([128, 4], f32)
    qt_p = persist.tile([128, 2, SEQ], bf16)             # Q^T head pairs
    kt_p = persist.tile([128, 2, SEQ], bf16)             # K^T head pairs
    v1 = persist.tile([128, NT, HG, DIM_HEAD + 1], bf16)  # V natural + ones col
    at_p = persist.tile([128, 2, SEQ], bf16)             # attn_out^T head pairs
    epst = persist.tile([128, 1], f32)
    ones64 = persist.tile([128, 64], bf16)

    nc.vector.memset(epst[:], EPS)
    nc.vector.memset(ones64[:], 1.0)
    nc.vector.memset(v1[:], 1.0)

    nc.sync.dma_start(bqk_sb[:], bqk_d.ap())
    nc.sync.dma_start(wq_sb[:], wq_d.ap().rearrange("(c p) j -> p c j", p=128))
    nc.sync.dma_start(wk_sb[:], wk_d.ap().rearrange("(c p) j -> p c j", p=128))
    nc.sync.dma_start(wv_sb[:], wv_d.ap().rearrange("(c p) j -> p c j", p=128))
    nc.sync.dma_start(
        wo_sb[:], wo_d.ap().rearrange("(jc p) (hf n) -> p jc hf n", p=128, n=512)
    )

    # ---- Stage A: LayerNorm, bounce x-hat through DRAM, DMA-transpose back ----
    with (
        tc.tile_pool(name="xin", bufs=3) as xin,
        tc.tile_pool(name="stat", bufs=3) as statp,
        tc.tile_pool(name="xh", bufs=3) as xhp,
        tc.tile_pool(name="xhd", bufs=1, space="DRAM") as xhd,
    ):
        xh_dram = xhd.tile([SEQ, DIM], bf16)
        for t in range(NT):
            xt = xin.tile([128, DIM], f32, tag="x")
            nc.sync.dma_start(xt[:], x_d.ap()[t * 128:(t + 1) * 128, :])
            bn = statp.tile([128, 12], f32, tag="bn")
            nc.vector.bn_stats(bn[:, 0:6], xt[:, 0:512])
            nc.vector.bn_stats(bn[:, 6:12], xt[:, 512:1024])
            mv = statp.tile([128, 2], f32, tag="mv")
            nc.vector.bn_aggr(mv[:], bn[:])
            std = statp.tile([128, 1], f32, tag="std")
            nc.scalar.activation(std[:], mv[:, 1:2], AF.Sqrt, bias=epst[:], scale=1.0)
            rstd = statp.tile([128, 1], f32, tag="rstd")
            nc.vector.reciprocal(rstd[:], std[:])
            nbias = statp.tile([128, 1], f32, tag="nb")
            nc.vector.scalar_tensor_tensor(
                nbias[:], mv[:, 0:1], -1.0, rstd[:], ALU.mult, ALU.mult
            )
            xh = xhp.tile([128, DIM], bf16, tag="xh")
            nc.scalar.activation(xh[:], xt[:], AF.Identity, bias=nbias[:], scale=rstd[:])
            nc.sync.dma_start(xh_dram[t * 128:(t + 1) * 128, :], xh[:])
        for c in range(NCK):
            nc.sync.dma_start_transpose(
                xhatT[:, c, :], xh_dram[:, c * 128:(c + 1) * 128]
            )

    # ---- Stage B: QKV^T projections + V natural ----
    with tc.tile_pool(name="psB", bufs=3, space="PSUM") as psB:
        for jc in range(2):  # head pair
            for qc in range(4):
                ps = psB.tile([128, 512], f32, tag="qk")
                for ci in range(NCK):
                    nc.tensor.matmul(
                        ps[:], wq_sb[:, ci, jc * 128:(jc + 1) * 128],
                        xhatT[:, ci, qc * 512:(qc + 1) * 512],
                        start=(ci == 0), stop=(ci == NCK - 1),
                    )
                nc.vector.tensor_scalar_add(
                    qt_p[:, jc, qc * 512:(qc + 1) * 512], ps[:], bqk_sb[:, jc:jc + 1]
                )
                ps = psB.tile([128, 512], f32, tag="qk")
                for ci in range(NCK):
                    nc.tensor.matmul(
                        ps[:], wk_sb[:, ci, jc * 128:(jc + 1) * 128],
                        xhatT[:, ci, qc * 512:(qc + 1) * 512],
                        start=(ci == 0), stop=(ci == NCK - 1),
                    )
                nc.vector.tensor_scalar_add(
                    kt_p[:, jc, qc * 512:(qc + 1) * 512], ps[:], bqk_sb[:, 2 + jc:3 + jc]
                )
        for t in range(NT):
            ps = psB.tile([128, 256], f32, tag="v")
            for ci in range(NCK):
                nc.tensor.matmul(
                    ps[:], xhatT[:, ci, t * 128:(t + 1) * 128], wv_sb[:, ci, :],
                    start=(ci == 0), stop=(ci == NCK - 1),
                )
            nc.vector.tensor_copy(
                v1[:, t, :, 0:DIM_HEAD], ps[:].rearrange("p (h d) -> p h d", h=HG)
            )

    # ---- Stage C: attention per head;  Stage D: output projection ----
    with (
        tc.tile_pool(name="psS", bufs=3, space="PSUM") as psS,
        tc.tile_pool(name="psAV", bufs=1, space="PSUM") as psAV,
        tc.tile_pool(name="exps", bufs=4) as expool,
        tc.tile_pool(name="nrm", bufs=2) as nrm,
        tc.tile_pool(name="osb", bufs=3) as osb,
    ):
        psD = psS
        # software-pipelined emission: S/exp for step i+1 is emitted before
        # the AV matmuls of step i, so the PE never waits on the ACT exp.
        all_avs = {}

        def emit_s_exp(h, kb):
            jc, lo = h // 2, (h % 2) * 64
            qlo = (kb // 2) * 256
            ex = expool.tile([128, SEQ], bf16, tag="ex", name=f"ex_h{h}_kb{kb}")
            for (plo, plen) in _pieces(qlo):
                sps = psS.tile([128, 512], f32, tag="s", name=f"s_h{h}_kb{kb}_{plo}")
                nc.tensor.matmul(
                    sps[:, 0:plen],
                    kt_p[lo:lo + 64, jc, kb * 128:(kb + 1) * 128],
                    qt_p[lo:lo + 64, jc, plo:plo + plen],
                    start=True, stop=True,
                )
                nc.scalar.activation(
                    ex[:, plo:plo + plen], sps[:, 0:plen], AF.Exp,
                    bias=0.0, scale=SCALE,
                )
            return ex

        def emit_av(h, kb, ex):
            jc, lo = h // 2, (h % 2) * 64
            f = kb // 2
            qlo = f * 256
            if kb == 0:
                all_avs[h] = [
                    psAV.tile([65, 512], f32, tag=f"av{qc}", name=f"av{qc}_h{h}")
                    for qc in range(4)
                ]
            avs = all_avs[h]
            for qc in range(f // 2, 4):
                cl = max(qlo - qc * 512, 0)
                nc.tensor.matmul(
                    avs[qc][:, cl:512],
                    v1[:, kb, h, :],
                    ex[:, qc * 512 + cl:(qc + 1) * 512],
                    start=(kb == 0), stop=(kb == 4 * qc + 3),
                )
                if kb == 4 * qc + 3:
                    emit_normalize(h, qc, avs[qc])

        def emit_normalize(h, qc, av):
            jc, lo = h // 2, (h % 2) * 64
            scr = nrm.tile([128, 512], bf16, tag="scr", name=f"scr_h{h}_q{qc}")
            nc.vector.tensor_copy(scr[64:65, :], av[64:65, :])
            # K=1 bf16 matmul replicates the denom row across 64 partitions
            bps = psS.tile([128, 512], f32, tag="s", name=f"bc_h{h}_q{qc}")
            nc.tensor.matmul(
                bps[0:64, :], ones64[64:65, :], scr[64:65, :],
                start=True, stop=True,
            )
            rec = nrm.tile([64, 512], f32, tag="rec", name=f"rec_h{h}_q{qc}")
            nc.vector.reciprocal_approx_fast(rec[:], bps[0:64, :])
            nc.vector.tensor_tensor(
                at_p[lo:lo + 64, jc, qc * 512:(qc + 1) * 512],
                av[0:64, :], rec[:], ALU.mult,
            )

        steps = [(h, kb) for h in range(HG) for kb in range(NT)]
        pend = None  # (h, kb, ex) awaiting AV emission
        for (h, kb) in steps:
            ex = emit_s_exp(h, kb)
            if pend is not None:
                emit_av(*pend)
            pend = (h, kb, ex)
        emit_av(*pend)
        for t in range(NT):
            ot = osb.tile([128, DIM], f32, tag="o")
            for hf in range(2):
                ops = psD.tile([128, 512], f32, tag="s", name=f"d_t{t}_f{hf}")
                for jc in range(2):
                    # pair tile spans both heads' inner dims: K=128 sums the pair
                    nc.tensor.matmul(
                        ops[:],
                        at_p[:, jc, t * 128:(t + 1) * 128],
                        wo_sb[:, jc, hf, :],
                        start=(jc == 0), stop=(jc == 1),
                    )
                if hf == 0:
                    nc.scalar.copy(ot[:, 0:512], ops[:])
                else:
                    nc.vector.tensor_copy(ot[:, 512:1024], ops[:])
            nc.sync.dma_start(out_d.ap()[t * 128:(t + 1) * 128, :], ot[:])


_CACHED_NC = None


def _get_nc():
    global _CACHED_NC
    if _CACHED_NC is None:
        _CACHED_NC = build_program()
    return _CACHED_NC


def shard_inputs(x, g, b, w_qkv, w_out, b_out):
    """Host-side prep: fold LN gain into w_qkv, slice per core."""
    bf16 = ml_dtypes.bfloat16
    g_f = np.asarray(g, np.float32).reshape(-1)
    b_f = np.asarray(b, np.float32).reshape(-1)
    w_qkv = np.asarray(w_qkv, np.float32)
    w_out = np.asarray(w_out, np.float32)
    wg = w_qkv * g_f[:, None]            # fold gain
    bqkv = b_f @ w_qkv                   # [3072] qkv bias from LN beta

    in_maps = []
    host_bias = np.zeros((BATCH, DIM), np.float32)
    for core in range(N_CORES):
        bb = core // 4
        hg = core % 4
        h0 = hg * HG
        c0 = h0 * DIM_HEAD
        sl = slice(c0, c0 + HG * DIM_HEAD)
        wq_c = wg[:, sl].astype(bf16)
        wk_c = wg[:, DIM + c0: DIM + c0 + HG * DIM_HEAD].astype(bf16)
        wv_c = wg[:, 2 * DIM + c0: 2 * DIM + c0 + HG * DIM_HEAD].astype(bf16)
        wo_c = w_out[sl, :].astype(bf16)
        bqk_c = np.zeros((128, 4), np.float32)
        bqk_c[:, 0] = bqkv[c0: c0 + 128]
        bqk_c[:, 1] = bqkv[c0 + 128: c0 + 256]
        bqk_c[:, 2] = bqkv[DIM + c0: DIM + c0 + 128]
        bqk_c[:, 3] = bqkv[DIM + c0 + 128: DIM + c0 + 256]
        # v-bias folds exactly into a constant output bias (attn rows sum to 1)
        bv_c = bqkv[2 * DIM + c0: 2 * DIM + c0 + HG * DIM_HEAD]
        host_bias[bb] += bv_c @ w_out[sl, :]
        in_maps.append({
            "x": np.ascontiguousarray(np.asarray(x, np.float32)[bb]),
            "wq": np.ascontiguousarray(wq_c),
            "wk": np.ascontiguousarray(wk_c),
            "wv": np.ascontiguousarray(wv_c),
            "wo": np.ascontiguousarray(wo_c),
            "bqk": bqk_c,
        })
    return in_maps, host_bias


def kernel(x, g, b, w_qkv, w_out, b_out, _results_hook=None):
    nc = _get_nc()
    in_maps, host_bias = shard_inputs(x, g, b, w_qkv, w_out, b_out)
    res = bass_utils.run_bass_kernel_spmd(nc, in_maps, core_ids=list(range(N_CORES)))
    if _results_hook is not None:
        _results_hook(res)
    out = np.zeros((BATCH, SEQ, DIM), np.float32)
    for core in range(N_CORES):
        out[core // 4] += res.results[core]["out"]
    out += host_bias[:, None, :]
    out += np.asarray(b_out, np.float32)[None, None, :]
    return out



# revision 5
# speedup vs baseline: 1.2100x; 1.2100x over previous
"""Trainium2 Bass kernel: frame-block-causal multi-head attention with LayerNorm.

Full module: LayerNorm(x) -> QKV proj -> 16-head block-causal attention
(8 frames x 256 patches) -> output projection.

Sharding: 8 cores = batch(2) x head-groups(4 heads each).  Each core gets its
batch's x and the weight column/row slices for its 4 heads, computes a partial
output [2048, 1024]; host sums the 4 partials per batch.  No collectives.

v2 design notes:
- LayerNorm output (bf16) is transposed per 128-row tile with an SBUF->SBUF
  XBAR transpose DMA into a per-tile-contiguous layout [p, t, c, n]; no DRAM
  bounce, so QKV matmuls start as soon as the first 4 tiles are normalized.
- Attention matmuls run the full 128x128 PE array: the S stationary (K^T) is
  zero-padded to K=128 (the other head's rows are zero, and the streamed Q^T
  pair tile contributes nothing through them); the AV stationary (V) is padded
  to M=128 with ones columns, which makes PSUM rows 64..127 of the AV output
  all equal to the softmax denominator - the normalization reciprocal reads
  them directly, replacing the old denominator-broadcast matmuls.  Full-array
  activity keeps the PE HAM un-throttled at 2.4 GHz (the v1 kernel's K=64/M=65
  matmuls ran the whole attention phase at half clock).
- Attention is query-block (qc) outer; S results for two key blocks land in a
  two-bank PSUM tile and are exponentiated by a single ACT instruction
  ([128, 1024]), halving ACT's per-instruction overhead.
- QKV for block qc+1 and the output projection for block qc-1 are emitted as
  filler between attention steps so the PE stays dense while ACT runs exp;
  output DMA streams during attention instead of as a tail.
- Output is written bf16 (the host sums the 4 partials per batch in fp32).

All matmuls run in bf16 with fp32 PSUM accumulation.  LayerNorm statistics are
computed in fp32.  g (LN gain) is folded into w_qkv on the host; LN beta's
qkv-bias is applied on-device for q/k and folded into a constant output bias
for v (softmax rows sum to 1).
"""

import numpy as np
import ml_dtypes

import concourse.bass as bass
import concourse.mybir as mybir
import concourse.tile as tile
from concourse import bacc, bass_utils

# ---- problem constants (hardcoded; kernel.py must be self-contained) ----
DIM = 1024
HEADS = 16
DIM_HEAD = 64
NUM_FRAMES = 8
PATCHES_PER_FRAME = 256
BATCH = 2
SEQ = NUM_FRAMES * PATCHES_PER_FRAME  # 2048
EPS = 1e-5
SCALE = DIM_HEAD ** -0.5  # 0.125

N_CORES = 8
HG = 4            # heads per core
NT = SEQ // 128   # 16 seq tiles of 128
NCK = DIM // 128  # 8 contraction chunks
NQC = 4           # query blocks of 512

dt = mybir.dt
AF = mybir.ActivationFunctionType
ALU = mybir.AluOpType


def build_program():
    nc = bacc.Bacc("TRN2", target_bir_lowering=False, debug=False)
    f32, bf16 = dt.float32, dt.bfloat16

    x_d = nc.dram_tensor("x", [SEQ, DIM], f32, kind="ExternalInput")
    wq_d = nc.dram_tensor("wq", [DIM, HG * DIM_HEAD], bf16, kind="ExternalInput")
    wk_d = nc.dram_tensor("wk", [DIM, HG * DIM_HEAD], bf16, kind="ExternalInput")
    wv_d = nc.dram_tensor("wv", [DIM, HG * DIM_HEAD], bf16, kind="ExternalInput")
    wo_d = nc.dram_tensor("wo", [HG * DIM_HEAD, DIM], bf16, kind="ExternalInput")
    bqk_d = nc.dram_tensor("bqk", [128, 4], f32, kind="ExternalInput")
    out_d = nc.dram_tensor("out", [SEQ, DIM], bf16, kind="ExternalOutput")

    with tile.TileContext(nc) as tc:
        with tc.tile_pool(name="persist", bufs=1) as persist:
            _build_body(tc, nc, persist, x_d, wq_d, wk_d, wv_d, wo_d, bqk_d, out_d)
    nc.compile()
    return nc


def _build_body(tc, nc, persist, x_d, wq_d, wk_d, wv_d, wo_d, bqk_d, out_d,
                dbg=None):
    f32, bf16 = dt.float32, dt.bfloat16

    # persistent SBUF tensors
    xhatT = persist.tile([128, NT, NCK, 128], bf16)   # [dim%128, tile, dim//128, tok%128]
    wq_sb = persist.tile([128, NCK, HG * DIM_HEAD], bf16)
    wk_sb = persist.tile([128, NCK, HG * DIM_HEAD], bf16)
    wv_sb = persist.tile([128, NCK, HG * DIM_HEAD], bf16)
    wo_sb = persist.tile([128, 2, 2, 512], bf16)      # [pair-row, jc, half, n]
    bqk_sb = persist.tile([128, 4], f32)
    qt_p = persist.tile([128, 2, SEQ], bf16)          # Q^T head pairs
    kt_pad = persist.tile([128, HG, SEQ], bf16)       # per-head K^T, other rows zero
    v1 = persist.tile([128, NT, HG, 128], bf16)       # V cols 0:64, ones cols 64:128
    at_p = persist.tile([128, 2, SEQ], bf16)          # attn_out^T head pairs
    epst = persist.tile([128, 1], f32)

    nc.vector.memset(epst[:], EPS)
    nc.vector.memset(v1[:], 1.0)
    # zero the complementary rows of each head's padded K^T
    for h in range(HG):
        if h % 2 == 0:
            nc.vector.memset(kt_pad[64:128, h, :], 0.0)
        else:
            nc.vector.memset(kt_pad[0:64, h, :], 0.0)

    nc.sync.dma_start(bqk_sb[:], bqk_d.ap())
    nc.sync.dma_start(wq_sb[:], wq_d.ap().rearrange("(c p) j -> p c j", p=128))
    nc.sync.dma_start(wk_sb[:], wk_d.ap().rearrange("(c p) j -> p c j", p=128))
    nc.sync.dma_start(wv_sb[:], wv_d.ap().rearrange("(c p) j -> p c j", p=128))
    nc.sync.dma_start(
        wo_sb[:], wo_d.ap().rearrange("(jc p) (hf n) -> p jc hf n", p=128, n=512)
    )

    with (
        tc.tile_pool(name="xin", bufs=3) as xin,
        tc.tile_pool(name="stat", bufs=3) as statp,
        tc.tile_pool(name="xh", bufs=3) as xhp,
        tc.tile_pool(name="psQ", bufs=2, space="PSUM") as psQ,
        tc.tile_pool(name="psS", bufs=2, space="PSUM") as psS,
        tc.tile_pool(name="psAV", bufs=2, space="PSUM") as psAV,
        tc.tile_pool(name="exps", bufs=3) as expool,
        tc.tile_pool(name="nrm", bufs=2) as nrm,
        tc.tile_pool(name="osb", bufs=2) as osb,
    ):
        # ---- Stage A: LayerNorm + per-tile SBUF->SBUF transpose ----
        for t in range(NT):
            xt = xin.tile([128, DIM], f32, tag="x")
            nc.sync.dma_start(xt[:], x_d.ap()[t * 128:(t + 1) * 128, :])
            bn = statp.tile([128, 12], f32, tag="bn")
            nc.vector.bn_stats(bn[:, 0:6], xt[:, 0:512])
            nc.vector.bn_stats(bn[:, 6:12], xt[:, 512:1024])
            mv = statp.tile([128, 2], f32, tag="mv")
            nc.vector.bn_aggr(mv[:], bn[:])
            std = statp.tile([128, 1], f32, tag="std")
            nc.scalar.activation(std[:], mv[:, 1:2], AF.Sqrt, bias=epst[:], scale=1.0)
            rstd = statp.tile([128, 1], f32, tag="rstd")
            nc.vector.reciprocal(rstd[:], std[:])
            nbias = statp.tile([128, 1], f32, tag="nb")
            nc.vector.scalar_tensor_tensor(
                nbias[:], mv[:, 0:1], -1.0, rstd[:], ALU.mult, ALU.mult
            )
            xh = xhp.tile([128, DIM], bf16, tag="xh")
            nc.scalar.activation(xh[:], xt[:], AF.Identity, bias=nbias[:], scale=rstd[:])
            # out[p, c, n] = xh[n, c*128+p]; dest is tile-t's contiguous block
            nc.scalar.dma_start_transpose(xhatT[:, t, :, :], xh[:])

        # ---- QKV / out-proj unit emitters (also used as attention fillers) ----
        def xT_rhs(ci, qc):
            return xhatT[:, 4 * qc:4 * qc + 4, ci, :]

        def emit_q(jc, qc):
            ps = psQ.tile([128, 512], f32, tag="qkv", name=f"q_{jc}_{qc}")
            for ci in range(NCK):
                nc.tensor.matmul(
                    ps[:], wq_sb[:, ci, jc * 128:(jc + 1) * 128], xT_rhs(ci, qc),
                    start=(ci == 0), stop=(ci == NCK - 1),
                )
            nc.vector.tensor_scalar_add(
                qt_p[:, jc, qc * 512:(qc + 1) * 512], ps[:], bqk_sb[:, jc:jc + 1]
            )

        def emit_k(jc, qc):
            ps = psQ.tile([128, 512], f32, tag="qkv", name=f"k_{jc}_{qc}")
            for ci in range(NCK):
                nc.tensor.matmul(
                    ps[:], wk_sb[:, ci, jc * 128:(jc + 1) * 128], xT_rhs(ci, qc),
                    start=(ci == 0), stop=(ci == NCK - 1),
                )
            sl = slice(qc * 512, (qc + 1) * 512)
            nc.vector.tensor_scalar_add(
                kt_pad[0:64, 2 * jc, sl], ps[0:64, :], bqk_sb[0:64, 2 + jc:3 + jc]
            )
            nc.vector.tensor_scalar_add(
                kt_pad[64:128, 2 * jc + 1, sl], ps[64:128, :],
                bqk_sb[64:128, 2 + jc:3 + jc]
            )

        def emit_v(t):
            ps = psQ.tile([128, 256], f32, tag="qkv", name=f"v_{t}")
            for ci in range(NCK):
                nc.tensor.matmul(
                    ps[:], xhatT[:, t, ci, :], wv_sb[:, ci, :],
                    start=(ci == 0), stop=(ci == NCK - 1),
                )
            nc.vector.tensor_copy(
                v1[:, t, :, 0:DIM_HEAD], ps[:].rearrange("p (h d) -> p h d", h=HG)
            )

        def emit_out(t):
            ot = osb.tile([128, DIM], bf16, tag="o", name=f"o_{t}")
            for hf in range(2):
                ps = psQ.tile([128, 512], f32, tag="qkv", name=f"d_{t}_{hf}")
                for jc in range(2):
                    nc.tensor.matmul(
                        ps[:], at_p[:, jc, t * 128:(t + 1) * 128],
                        wo_sb[:, jc, hf, :],
                        start=(jc == 0), stop=(jc == 1),
                    )
                nc.vector.tensor_copy(ot[:, hf * 512:(hf + 1) * 512], ps[:])
            nc.sync.dma_start(out_d.ap()[t * 128:(t + 1) * 128, :], ot[:])

        def qkv_units(qc):
            us = []
            for jc in range(2):
                us.append(lambda jc=jc: emit_q(jc, qc))
                us.append(lambda jc=jc: emit_k(jc, qc))
            for t in range(4 * qc, 4 * qc + 4):
                us.append(lambda t=t: emit_v(t))
            return us

        def out_units(qc):
            return [lambda t=t: emit_out(t) for t in range(4 * qc, 4 * qc + 4)]

        # ---- attention steps for one (h, qc): groups of 2 key blocks ----
        # kbs 0..4qc+1 attend all 512 queries of qc; kbs 4qc+2..4qc+3 only the
        # last 256 (frame-causal).  The last AV also triggers normalization.
        av_tiles = {}

        def emit_s_exp(h, qc, g):
            jc = h // 2
            half = (g == 2 * qc + 1)
            lo = 256 if half else 0
            sg = psS.tile([128, 2, 512], f32, tag="s", name=f"s_h{h}_q{qc}_g{g}")
            ex = expool.tile([128, 2, 512], bf16, tag="ex", name=f"ex_h{h}_q{qc}_g{g}")
            for j in range(2):
                kb = 2 * g + j
                nc.tensor.matmul(
                    sg[:, j, lo:512],
                    kt_pad[:, h, kb * 128:(kb + 1) * 128],
                    qt_p[:, jc, qc * 512 + lo:(qc + 1) * 512],
                    start=True, stop=True,
                )
            nc.scalar.activation(
                ex[:, :, lo:512], sg[:, :, lo:512], AF.Exp, bias=0.0, scale=SCALE
            )
            return ex

        def emit_av(h, qc, g, ex):
            jc = h // 2
            half = (g == 2 * qc + 1)
            lo = 256 if half else 0
            if g == 0:
                av_tiles[(h, qc)] = psAV.tile(
                    [128, 512], f32, tag="av", name=f"av_h{h}_q{qc}"
                )
            av = av_tiles[(h, qc)]
            for j in range(2):
                kb = 2 * g + j
                nc.tensor.matmul(
                    av[:, lo:512], v1[:, kb, h, :], ex[:, j, lo:512],
                    start=(kb == 0), stop=(kb == 4 * qc + 3),
                )
            if half:
                # rows 64:128 of av all hold the softmax denominator; shift a
                # copy to base partition 0 (reciprocal_approx_fast is only
                # correct at base 0)
                scr = nrm.tile([64, 512], f32, tag="scr", name=f"scr_h{h}_q{qc}")
                nc.vector.tensor_copy(scr[:], av[64:128, :])
                rec = nrm.tile([64, 512], f32, tag="rec", name=f"rec_h{h}_q{qc}")
                nc.vector.reciprocal_approx_fast(rec[:], scr[:])
                plo = (h % 2) * 64
                nc.vector.tensor_tensor(
                    at_p[plo:plo + 64, jc, qc * 512:(qc + 1) * 512],
                    av[0:64, :], rec[:], ALU.mult,
                )

        # ---- emission: QKV(0), then per qc attention with fillers ----
        for u in qkv_units(0):
            u()
        for qc in range(NQC):
            steps = [(h, g) for h in range(HG) for g in range(2 * qc + 2)]
            fillers = []
            if qc + 1 < NQC:
                fillers += qkv_units(qc + 1)
            if qc >= 1:
                fillers += out_units(qc - 1)
            n_f, n_s = len(fillers), len(steps)
            fi = 0
            pend = None
            for si, (h, g) in enumerate(steps):
                ex = emit_s_exp(h, qc, g)
                if pend is not None:
                    emit_av(*pend)
                pend = (h, qc, g, ex)
                while fi < n_f and (si + 1) * n_f >= (fi + 1) * n_s:
                    fillers[fi]()
                    fi += 1
            emit_av(*pend)
            while fi < n_f:
                fillers[fi]()
                fi += 1
        for u in out_units(NQC - 1):
            u()
        if dbg is not None:
            for name, sb in [("dbg_qt", qt_p), ("dbg_kt", kt_pad), ("dbg_v1", v1),
                             ("dbg_at", at_p), ("dbg_xt", xhatT)]:
                if name in dbg:
                    nc.sync.dma_start(dbg[name].ap(), sb[:])


_CACHED_NC = None


def _get_nc():
    global _CACHED_NC
    if _CACHED_NC is None:
        _CACHED_NC = build_program()
    return _CACHED_NC


def shard_inputs(x, g, b, w_qkv, w_out, b_out):
    """Host-side prep: fold LN gain into w_qkv, slice per core."""
    bf16 = ml_dtypes.bfloat16
    g_f = np.asarray(g, np.float32).reshape(-1)
    b_f = np.asarray(b, np.float32).reshape(-1)
    w_qkv = np.asarray(w_qkv, np.float32)
    w_out = np.asarray(w_out, np.float32)
    wg = w_qkv * g_f[:, None]            # fold gain
    bqkv = b_f @ w_qkv                   # [3072] qkv bias from LN beta

    in_maps = []
    host_bias = np.zeros((BATCH, DIM), np.float32)
    for core in range(N_CORES):
        bb = core // 4
        hg = core % 4
        h0 = hg * HG
        c0 = h0 * DIM_HEAD
        sl = slice(c0, c0 + HG * DIM_HEAD)
        wq_c = wg[:, sl].astype(bf16)
        wk_c = wg[:, DIM + c0: DIM + c0 + HG * DIM_HEAD].astype(bf16)
        wv_c = wg[:, 2 * DIM + c0: 2 * DIM + c0 + HG * DIM_HEAD].astype(bf16)
        wo_c = w_out[sl, :].astype(bf16)
        bqk_c = np.zeros((128, 4), np.float32)
        bqk_c[:, 0] = bqkv[c0: c0 + 128]
        bqk_c[:, 1] = bqkv[c0 + 128: c0 + 256]
        bqk_c[:, 2] = bqkv[DIM + c0: DIM + c0 + 128]
        bqk_c[:, 3] = bqkv[DIM + c0 + 128: DIM + c0 + 256]
        # v-bias folds exactly into a constant output bias (attn rows sum to 1)
        bv_c = bqkv[2 * DIM + c0: 2 * DIM + c0 + HG * DIM_HEAD]
        host_bias[bb] += bv_c @ w_out[sl, :]
        in_maps.append({
            "x": np.ascontiguousarray(np.asarray(x, np.float32)[bb]),
            "wq": np.ascontiguousarray(wq_c),
            "wk": np.ascontiguousarray(wk_c),
            "wv": np.ascontiguousarray(wv_c),
            "wo": np.ascontiguousarray(wo_c),
            "bqk": bqk_c,
        })
    return in_maps, host_bias


def kernel(x, g, b, w_qkv, w_out, b_out, _results_hook=None):
    nc = _get_nc()
    in_maps, host_bias = shard_inputs(x, g, b, w_qkv, w_out, b_out)
    res = bass_utils.run_bass_kernel_spmd(nc, in_maps, core_ids=list(range(N_CORES)))
    if _results_hook is not None:
        _results_hook(res)
    out = np.zeros((BATCH, SEQ, DIM), np.float32)
    for core in range(N_CORES):
        out[core // 4] += np.asarray(res.results[core]["out"], np.float32)
    out += host_bias[:, None, :]
    out += np.asarray(b_out, np.float32)[None, None, :]
    return out


# revision 8
# speedup vs baseline: 1.6753x; 1.3846x over previous
"""Trainium2 Bass kernel: frame-block-causal multi-head attention with LayerNorm.

Full module: LayerNorm(x) -> QKV proj -> 16-head block-causal attention
(8 frames x 256 patches) -> output projection.

Sharding: 8 cores = batch(2) x head-groups(4 heads each).  Each core gets its
batch's x and the weight column/row slices for its 4 heads, computes a partial
output [2048, 1024]; host sums the 4 partials per batch.  No collectives.

v2 design notes:
- LayerNorm output (bf16) is transposed per 128-row tile with an SBUF->SBUF
  XBAR transpose DMA into a per-tile-contiguous layout [p, t, c, n]; no DRAM
  bounce, so QKV matmuls start as soon as the first 4 tiles are normalized.
- Attention matmuls run the full 128x128 PE array: the S stationary (K^T) is
  zero-padded to K=128 (the other head's rows are zero, and the streamed Q^T
  pair tile contributes nothing through them); the AV stationary (V) is padded
  to M=128 with ones columns, which makes PSUM rows 64..127 of the AV output
  all equal to the softmax denominator - the normalization reciprocal reads
  them directly, replacing the old denominator-broadcast matmuls.  Full-array
  activity keeps the PE HAM un-throttled at 2.4 GHz (the v1 kernel's K=64/M=65
  matmuls ran the whole attention phase at half clock).
- Attention is query-block (qc) outer; S results for two key blocks land in a
  two-bank PSUM tile and are exponentiated by a single ACT instruction
  ([128, 1024]), halving ACT's per-instruction overhead.
- QKV for block qc+1 and the output projection for block qc-1 are emitted as
  filler between attention steps so the PE stays dense while ACT runs exp;
  output DMA streams during attention instead of as a tail.
- Output is written bf16 (the host sums the 4 partials per batch in fp32).

All matmuls run in bf16 with fp32 PSUM accumulation.  LayerNorm statistics are
computed in fp32.  g (LN gain) is folded into w_qkv on the host; LN beta's
qkv-bias is applied on-device for q/k and folded into a constant output bias
for v (softmax rows sum to 1).
"""

import numpy as np
import ml_dtypes

import concourse.bass as bass
import concourse.mybir as mybir
import concourse.tile as tile
from concourse import bacc, bass_utils

# ---- problem constants (hardcoded; kernel.py must be self-contained) ----
DIM = 1024
HEADS = 16
DIM_HEAD = 64
NUM_FRAMES = 8
PATCHES_PER_FRAME = 256
BATCH = 2
SEQ = NUM_FRAMES * PATCHES_PER_FRAME  # 2048
EPS = 1e-5
SCALE = DIM_HEAD ** -0.5  # 0.125

N_CORES = 8
HG = 4            # heads per core
NT = SEQ // 128   # 16 seq tiles of 128
NCK = DIM // 128  # 8 contraction chunks
NQC = 4           # query blocks of 512

dt = mybir.dt
AF = mybir.ActivationFunctionType
ALU = mybir.AluOpType


def build_program():
    nc = bacc.Bacc("TRN2", target_bir_lowering=False, debug=False)
    f32, bf16 = dt.float32, dt.bfloat16

    x_d = nc.dram_tensor("x", [SEQ, DIM], f32, kind="ExternalInput")
    wq_d = nc.dram_tensor("wq", [DIM, HG * DIM_HEAD], bf16, kind="ExternalInput")
    wk_d = nc.dram_tensor("wk", [DIM, HG * DIM_HEAD], bf16, kind="ExternalInput")
    wv_d = nc.dram_tensor("wv", [DIM, HG * DIM_HEAD], bf16, kind="ExternalInput")
    wo_d = nc.dram_tensor("wo", [HG * DIM_HEAD, DIM], bf16, kind="ExternalInput")
    bqk_d = nc.dram_tensor("bqk", [128, 4], f32, kind="ExternalInput")
    out_d = nc.dram_tensor("out", [SEQ, DIM], bf16, kind="ExternalOutput")

    with tile.TileContext(nc) as tc:
        with tc.tile_pool(name="persist", bufs=1) as persist:
            _build_body(tc, nc, persist, x_d, wq_d, wk_d, wv_d, wo_d, bqk_d, out_d)
    nc.compile()
    return nc


def _build_body(tc, nc, persist, x_d, wq_d, wk_d, wv_d, wo_d, bqk_d, out_d,
                dbg=None):
    f32, bf16 = dt.float32, dt.bfloat16

    # persistent SBUF tensors
    xhatT = persist.tile([128, NT, NCK, 128], bf16)   # [dim%128, tile, dim//128, tok%128]
    wq_sb = persist.tile([128, NCK, HG * DIM_HEAD], bf16)
    wk_sb = persist.tile([128, NCK, HG * DIM_HEAD], bf16)
    wv_sb = persist.tile([128, NCK, HG * DIM_HEAD], bf16)
    wo_sb = persist.tile([128, 2, 2, 512], bf16)      # [pair-row, jc, half, n]
    bqk_sb = persist.tile([128, 4], f32)
    qt_p = persist.tile([128, 2, SEQ], bf16)          # Q^T head pairs
    kt_pad = persist.tile([128, HG, SEQ], bf16)       # per-head K^T, other rows zero
    v1 = persist.tile([128, NT, HG, 128], bf16)       # V cols 0:64, ones cols 64:128
    at_p = persist.tile([128, 2, SEQ], bf16)          # attn_out^T head pairs
    epst = persist.tile([128, 1], f32)

    nc.vector.memset(epst[:], EPS)
    # big memsets go on the otherwise-idle gpsimd so they don't delay the
    # LayerNorm chain on the vector engine
    nc.gpsimd.memset(v1[:], 1.0)
    # zero the complementary rows of each head's padded K^T
    for h in range(HG):
        if h % 2 == 0:
            nc.gpsimd.memset(kt_pad[64:128, h, :], 0.0)
        else:
            nc.gpsimd.memset(kt_pad[0:64, h, :], 0.0)

    nc.sync.dma_start(bqk_sb[:], bqk_d.ap())
    nc.sync.dma_start(wq_sb[:], wq_d.ap().rearrange("(c p) j -> p c j", p=128))
    nc.sync.dma_start(wk_sb[:], wk_d.ap().rearrange("(c p) j -> p c j", p=128))
    nc.sync.dma_start(wv_sb[:], wv_d.ap().rearrange("(c p) j -> p c j", p=128))
    nc.sync.dma_start(
        wo_sb[:], wo_d.ap().rearrange("(jc p) (hf n) -> p jc hf n", p=128, n=512)
    )

    with (
        tc.tile_pool(name="xin", bufs=3) as xin,
        tc.tile_pool(name="stat", bufs=3) as statp,
        tc.tile_pool(name="xh", bufs=3) as xhp,
        tc.tile_pool(name="psQ", bufs=2, space="PSUM") as psQ,
        tc.tile_pool(name="psS", bufs=2, space="PSUM") as psS,
        tc.tile_pool(name="psAV", bufs=2, space="PSUM") as psAV,
        tc.tile_pool(name="exps", bufs=3) as expool,
        tc.tile_pool(name="nrm", bufs=2) as nrm,
        tc.tile_pool(name="osb", bufs=2) as osb,
    ):
        # ---- Stage A: LayerNorm + per-tile SBUF->SBUF transpose ----
        # x is loaded two tiles per DMA; emitted in 4-tile sections interleaved
        # with QKV/attention (see the bottom of this function)
        def emit_ln(t0):
            for tp in range(t0 // 2, t0 // 2 + 2):
                xt = xin.tile([128, 2, DIM], f32, tag="x", name=f"x_{tp}")
                nc.sync.dma_start(
                    xt[:],
                    x_d.ap()[tp * 256:(tp + 1) * 256, :]
                    .rearrange("(t p) d -> p t d", p=128),
                )
                for i in range(2):
                    t = 2 * tp + i
                    bn = statp.tile([128, 12], f32, tag="bn")
                    nc.vector.bn_stats(bn[:, 0:6], xt[:, i, 0:512])
                    nc.vector.bn_stats(bn[:, 6:12], xt[:, i, 512:1024])
                    mv = statp.tile([128, 2], f32, tag="mv")
                    nc.vector.bn_aggr(mv[:], bn[:])
                    std = statp.tile([128, 1], f32, tag="std")
                    nc.scalar.activation(
                        std[:], mv[:, 1:2], AF.Sqrt, bias=epst[:], scale=1.0
                    )
                    rstd = statp.tile([128, 1], f32, tag="rstd")
                    nc.vector.reciprocal(rstd[:], std[:])
                    nbias = statp.tile([128, 1], f32, tag="nb")
                    nc.vector.scalar_tensor_tensor(
                        nbias[:], mv[:, 0:1], -1.0, rstd[:], ALU.mult, ALU.mult
                    )
                    xh = xhp.tile([128, DIM], bf16, tag="xh")
                    nc.scalar.activation(
                        xh[:], xt[:, i, :], AF.Identity, bias=nbias[:], scale=rstd[:]
                    )
                    # out[p, c, n] = xh[n, c*128+p]; dest is tile-t's block
                    nc.sync.dma_start_transpose(xhatT[:, t, :, :], xh[:])

        # ---- QKV / out-proj unit emitters (also used as attention fillers) ----
        def xT_rhs(ci, qc):
            return xhatT[:, 4 * qc:4 * qc + 4, ci, :]

        def emit_q(jc, qc):
            ps = psQ.tile([128, 512], f32, tag="qkv", name=f"q_{jc}_{qc}")
            for ci in range(NCK):
                nc.tensor.matmul(
                    ps[:], wq_sb[:, ci, jc * 128:(jc + 1) * 128], xT_rhs(ci, qc),
                    start=(ci == 0), stop=(ci == NCK - 1),
                )
            nc.vector.tensor_scalar_add(
                qt_p[:, jc, qc * 512:(qc + 1) * 512], ps[:], bqk_sb[:, jc:jc + 1]
            )

        def emit_k(jc, qc):
            ps = psQ.tile([128, 512], f32, tag="qkv", name=f"k_{jc}_{qc}")
            for ci in range(NCK):
                nc.tensor.matmul(
                    ps[:], wk_sb[:, ci, jc * 128:(jc + 1) * 128], xT_rhs(ci, qc),
                    start=(ci == 0), stop=(ci == NCK - 1),
                )
            sl = slice(qc * 512, (qc + 1) * 512)
            nc.vector.tensor_scalar_add(
                kt_pad[0:64, 2 * jc, sl], ps[0:64, :], bqk_sb[0:64, 2 + jc:3 + jc]
            )
            nc.vector.tensor_scalar_add(
                kt_pad[64:128, 2 * jc + 1, sl], ps[64:128, :],
                bqk_sb[64:128, 2 + jc:3 + jc]
            )

        def emit_v(t):
            ps = psQ.tile([128, 256], f32, tag="qkv", name=f"v_{t}")
            for ci in range(NCK):
                nc.tensor.matmul(
                    ps[:], xhatT[:, t, ci, :], wv_sb[:, ci, :],
                    start=(ci == 0), stop=(ci == NCK - 1),
                )
            nc.vector.tensor_copy(
                v1[:, t, :, 0:DIM_HEAD], ps[:].rearrange("p (h d) -> p h d", h=HG)
            )

        def emit_out(t):
            ot = osb.tile([128, DIM], bf16, tag="o", name=f"o_{t}")
            for hf in range(2):
                ps = psQ.tile([128, 512], f32, tag="qkv", name=f"d_{t}_{hf}")
                for jc in range(2):
                    nc.tensor.matmul(
                        ps[:], at_p[:, jc, t * 128:(t + 1) * 128],
                        wo_sb[:, jc, hf, :],
                        start=(jc == 0), stop=(jc == 1),
                    )
                nc.vector.tensor_copy(ot[:, hf * 512:(hf + 1) * 512], ps[:])
            nc.sync.dma_start(out_d.ap()[t * 128:(t + 1) * 128, :], ot[:])

        def qkv_units(qc):
            us = []
            for jc in range(2):
                us.append(lambda jc=jc: emit_q(jc, qc))
                us.append(lambda jc=jc: emit_k(jc, qc))
            for t in range(4 * qc, 4 * qc + 4):
                us.append(lambda t=t: emit_v(t))
            return us

        def out_units(qc):
            return [lambda t=t: emit_out(t) for t in range(4 * qc, 4 * qc + 4)]

        # ---- attention steps for one (h, qc): groups of 2 key blocks ----
        # kbs 0..4qc+1 attend all 512 queries of qc; kbs 4qc+2..4qc+3 only the
        # last 256 (frame-causal).  The last AV also triggers normalization.
        av_tiles = {}

        def emit_s_exp(h, qc, g):
            jc = h // 2
            half = (g == 2 * qc + 1)
            lo = 256 if half else 0
            sg = psS.tile([128, 2, 512], f32, tag="s", name=f"s_h{h}_q{qc}_g{g}")
            ex = expool.tile([128, 2, 512], bf16, tag="ex", name=f"ex_h{h}_q{qc}_g{g}")
            for j in range(2):
                kb = 2 * g + j
                nc.tensor.matmul(
                    sg[:, j, lo:512],
                    kt_pad[:, h, kb * 128:(kb + 1) * 128],
                    qt_p[:, jc, qc * 512 + lo:(qc + 1) * 512],
                    start=True, stop=True,
                )
            nc.scalar.activation(
                ex[:, :, lo:512], sg[:, :, lo:512], AF.Exp, bias=0.0, scale=SCALE
            )
            return ex

        def emit_av(h, qc, g, ex):
            jc = h // 2
            half = (g == 2 * qc + 1)
            lo = 256 if half else 0
            if g == 0:
                av_tiles[(h, qc)] = psAV.tile(
                    [128, 512], f32, tag="av", name=f"av_h{h}_q{qc}"
                )
            av = av_tiles[(h, qc)]
            for j in range(2):
                kb = 2 * g + j
                nc.tensor.matmul(
                    av[:, lo:512], v1[:, kb, h, :], ex[:, j, lo:512],
                    start=(kb == 0), stop=(kb == 4 * qc + 3),
                )
            if half:
                # rows 64:128 of av all hold the softmax denominator; shift a
                # copy to base partition 0 (reciprocal_approx_fast is only
                # correct at base 0)
                scr = nrm.tile([64, 512], f32, tag="scr", name=f"scr_h{h}_q{qc}")
                nc.vector.tensor_copy(scr[:], av[64:128, :])
                rec = nrm.tile([64, 512], f32, tag="rec", name=f"rec_h{h}_q{qc}")
                nc.vector.reciprocal_approx_fast(rec[:], scr[:])
                plo = (h % 2) * 64
                nc.vector.tensor_tensor(
                    at_p[plo:plo + 64, jc, qc * 512:(qc + 1) * 512],
                    av[0:64, :], rec[:], ALU.mult,
                )

        # ---- emission: LN sections feed QKV(qc); attention(qc) interleaves
        # QKV(qc+1) and out-proj(qc-1) as PE filler while ACT runs exp ----
        emit_ln(0)
        for u in qkv_units(0):
            u()
        emit_ln(4)
        for qc in range(NQC):
            steps = [(h, g) for h in range(HG) for g in range(2 * qc + 2)]
            fillers = []
            if qc == 0:
                fillers.append(lambda: emit_ln(8))
            elif qc == 1:
                fillers.append(lambda: emit_ln(12))
            if qc + 1 < NQC:
                fillers += qkv_units(qc + 1)
            if qc >= 1:
                fillers += out_units(qc - 1)
            n_f, n_s = len(fillers), len(steps)
            fi = 0
            pend = None
            for si, (h, g) in enumerate(steps):
                ex = emit_s_exp(h, qc, g)
                if pend is not None:
                    emit_av(*pend)
                pend = (h, qc, g, ex)
                while fi < n_f and (si + 1) * n_f >= (fi + 1) * n_s:
                    fillers[fi]()
                    fi += 1
            emit_av(*pend)
            while fi < n_f:
                fillers[fi]()
                fi += 1
        for u in out_units(NQC - 1):
            u()
        if dbg is not None:
            for name, sb in [("dbg_qt", qt_p), ("dbg_kt", kt_pad), ("dbg_v1", v1),
                             ("dbg_at", at_p), ("dbg_xt", xhatT)]:
                if name in dbg:
                    nc.sync.dma_start(dbg[name].ap(), sb[:])


_CACHED_NC = None


def _get_nc():
    global _CACHED_NC
    if _CACHED_NC is None:
        _CACHED_NC = build_program()
    return _CACHED_NC


def shard_inputs(x, g, b, w_qkv, w_out, b_out):
    """Host-side prep: fold LN gain into w_qkv, slice per core."""
    bf16 = ml_dtypes.bfloat16
    g_f = np.asarray(g, np.float32).reshape(-1)
    b_f = np.asarray(b, np.float32).reshape(-1)
    w_qkv = np.asarray(w_qkv, np.float32)
    w_out = np.asarray(w_out, np.float32)
    wg = w_qkv * g_f[:, None]            # fold gain
    bqkv = b_f @ w_qkv                   # [3072] qkv bias from LN beta

    in_maps = []
    host_bias = np.zeros((BATCH, DIM), np.float32)
    for core in range(N_CORES):
        bb = core // 4
        hg = core % 4
        h0 = hg * HG
        c0 = h0 * DIM_HEAD
        sl = slice(c0, c0 + HG * DIM_HEAD)
        wq_c = wg[:, sl].astype(bf16)
        wk_c = wg[:, DIM + c0: DIM + c0 + HG * DIM_HEAD].astype(bf16)
        wv_c = wg[:, 2 * DIM + c0: 2 * DIM + c0 + HG * DIM_HEAD].astype(bf16)
        wo_c = w_out[sl, :].astype(bf16)
        bqk_c = np.zeros((128, 4), np.float32)
        bqk_c[:, 0] = bqkv[c0: c0 + 128]
        bqk_c[:, 1] = bqkv[c0 + 128: c0 + 256]
        bqk_c[:, 2] = bqkv[DIM + c0: DIM + c0 + 128]
        bqk_c[:, 3] = bqkv[DIM + c0 + 128: DIM + c0 + 256]
        # v-bias folds exactly into a constant output bias (attn rows sum to 1)
        bv_c = bqkv[2 * DIM + c0: 2 * DIM + c0 + HG * DIM_HEAD]
        host_bias[bb] += bv_c @ w_out[sl, :]
        in_maps.append({
            "x": np.ascontiguousarray(np.asarray(x, np.float32)[bb]),
            "wq": np.ascontiguousarray(wq_c),
            "wk": np.ascontiguousarray(wk_c),
            "wv": np.ascontiguousarray(wv_c),
            "wo": np.ascontiguousarray(wo_c),
            "bqk": bqk_c,
        })
    return in_maps, host_bias


def kernel(x, g, b, w_qkv, w_out, b_out, _results_hook=None):
    nc = _get_nc()
    in_maps, host_bias = shard_inputs(x, g, b, w_qkv, w_out, b_out)
    res = bass_utils.run_bass_kernel_spmd(nc, in_maps, core_ids=list(range(N_CORES)))
    if _results_hook is not None:
        _results_hook(res)
    out = np.zeros((BATCH, SEQ, DIM), np.float32)
    for core in range(N_CORES):
        out[core // 4] += np.asarray(res.results[core]["out"], np.float32)
    out += host_bias[:, None, :]
    out += np.asarray(b_out, np.float32)[None, None, :]
    return out
